# revision 31
# baseline (speedup 1.0000x reference)
"""Block-local sparse attention (LSG-style) on 8 TRN2 NeuronCores.

Sharding: the 32 (n, h) pairs are split 4-per-core (data/head parallel, no
collectives). Host-side numpy prep re-lays-out the inputs so the device
kernel needs no transposes, all bf16:

  - qt : Q^T per head, zero-padded to [128, T] on the host.
  - lkt/skt/gkt: local/sparse/global K^T, token-padded with zeros and
    row-padded to 128 partitions (uniform 128-row PE tile shapes keep the
    HAM activity monitor from down-clocking the tensor engine).
  - lv/sv/gv: V with a ones column appended (col 64), chunked [128, c, 65],
    every row scaled by exp(mask): softmax(QK/8 + m) @ V is computed as
    sum_t exp(s_t) e^{m_t} [V_t, 1]; the divide by the accumulated last
    column happens ON THE HOST (output is unnormalized ctx|Z), so the DVE
    has no normalization work on device.
    sv holds 4 phase-shifted copies so the 32-token-granular sparse windows
    always start at partition 0.

The device processes query-block PAIRS: 9 score matmuls per pair into a
3-bank PSUM region [128, 1536].  exp is split across two engines at PSUM
BANK granularity (an engine reading a PSUM bank while another engine
touches the same bank hard-crashes the device, so the split must be
bank-aligned):
  - ACT: one activation exp(S/8) over cols 0:1024 (banks 0-1: sparse,
    global, local b+1) -> bf16 pp.  ~1.0-1.1 us/pair; the limiter.
  - DVE: Schraudolph exp over cols 1024:1536 (bank 2: local b+2, b, b+3)
    via one tensor_scalar: bf16 bits of exp(x/8) ~= int16(x*A + B)
    (rel err ~2%/element on ~36% of context tokens; end-to-end rel err
    ~1.1e-2 vs the 2e-2 gate).  ~0.68 us/pair.
Then 12 PV matmuls (6 per block, N=65) accumulate [q, V|Z] into pv cols
0:65 / 65:130, and ONE 66.5KB DMA per pair stores pv straight from PSUM
(PV(p+2) is gated on store(p) completion so the PE never writes a bank a
store is still reading).

DMA queues: each queue costs a flat ~0.6us per DMA instruction
(descriptor generation), so queue ASSIGNMENT matters more than bytes:
  - sync/HWDGE queue: the startup-critical phase-0 + gkt, then ONLY the
    64 pair stores (a store waiting at the FIFO head can therefore never
    delay input loads - that head-of-line blocking caused slot-boundary
    PE starvation and HAM re-throttles in earlier versions).
  - gpsimd/SWDGE queue: everything else - gv (first, to swallow any
    one-time SWDGE setup), the rest of slot 0 in 3 column-phases, then
    slots 1-3 as 5 whole-slot transfers each, paced by pe_v.

Known hardware landmine (cost several device wedges in a previous
session): concurrent same-PSUM-bank access by two engines (ACT read + DVE
read, or PE matmul write + DVE read) hard-crashes the device
(NRT_EXEC_UNIT_UNRECOVERABLE).  The bank-aligned ACT/DVE split plus the
act/sch/st gates keep every bank single-toucher while in use.
"""

from contextlib import ExitStack

import numpy as np

import concourse.bass as bass
import concourse.mybir as mybir
from concourse.bass_utils import run_bass_kernel_spmd

N, H, T, D = 2, 16, 4096, 64
B = 128          # query block
NB = T // B      # 32
G = 64           # global tokens
TSP = T // 4     # sparse tokens (1024)
NH = N * H       # 32
NCORES = 8
SL = NH // NCORES  # 4 heads per core
NP = SL * NB // 2  # 64 block-pairs per core
PPS = NB // 2      # 16 pairs per slot

LKT_W = T + 2 * B            # 4352 padded local tokens
SKT_W = TSP + 320            # 1344 padded sparse tokens
LV_C = LKT_W // 128          # 34 local V chunks
SV_C = 11                    # sparse V chunks per phase

F32 = mybir.dt.float32
BF16 = mybir.dt.bfloat16
GE = "sem-ge"

# column layout of the per-pair score/prob tile [128, 1536] (3 PSUM banks;
# regions never cross a 512-col bank boundary).  Cols 0:1024 (banks 0-1)
# are exp'd by ACT; cols 1024:1536 (bank 2) by the DVE Schraudolph trick.
C_SP1A, C_SP1B = 0, 128
C_SP2A, C_SP2B = 256, 384
C_G = 512        # 256 wide: q of both blocks
C_LOC1 = 768     # 256 wide: local chunk b+1, both blocks
C_LOC2 = 1024    # 256 wide: local chunk b+2, both blocks (DVE exp)
C_LOC0 = 1280    # 128: local chunk b, block A only (DVE exp)
C_LOC3 = 1408    # 128: local chunk b+3, block B only (DVE exp)

# Schraudolph constants: bf16 bits of exp(x*0.125) ~= int16(x*SCH_A + SCH_B)
SCH_A = float(128 * 1.4426950408889634 * 0.125)
SCH_B = 16256.0 - 0.057 * 128.0

# Input staging.  Slot 0 loads in 4 column-phases (compute starts after
# phase 0); slots 1-3 load each tensor with ONE whole-slot DMA.
# phase = (qt-range, lkt-range, skt-range, lv-chunk-range, sv-chunk-range,
#          gate_hb)  — gate_hb is the first pair needing it.
PHASES_S0 = (
    ((0, 512), (0, 768), (0, 448), (0, 8), (0, 3), 0),
    ((512, 1024), (768, 1280), (448, 576), (8, 12), (3, 4), 2),
    ((1024, 2560), (1280, 2816), (576, 960), (12, 24), (4, 7), 4),
    ((2560, 4096), (2816, LKT_W), (960, SKT_W), (24, LV_C), (7, SV_C), 10),
)
PHASE_FULL = ((0, T), (0, LKT_W), (0, SKT_W), (0, LV_C), (0, SV_C), 0)


def _slot_phases(s):
    return PHASES_S0 if s == 0 else (PHASE_FULL,)


# A semaphore wait is only race-free at the end of a maximal run of
# consecutive instructions updating that semaphore, so consecutive phases
# alternate between two semaphores per slot parity: di[parity][phase_seq%2].
# DI_SEM[(s, k)] = (parity, alt) and DI_GATE[(s, hb)] = (parity, alt, value).
DI_SEM = {}
DI_GATE = {}
for _u in range(2):
    _cum = [0, 0]
    _seq = 0
    for _s in range(_u, SL, 2):
        for _k, _ph in enumerate(_slot_phases(_s)):
            _alt = _seq % 2
            _cum[_alt] += 16 * 5
            DI_SEM[(_s, _k)] = (_u, _alt)
            DI_GATE[(_s, _ph[5])] = (_u, _alt, _cum[_alt])
            _seq += 1


def _build_bass():
    nc = bass.Bass("TRN2", num_devices=NCORES, debug=False)

    qt = nc.dram_tensor("qt", [SL, 128, T], BF16, kind="ExternalInput")
    lkt = nc.dram_tensor("lkt", [SL, 128, LKT_W], BF16, kind="ExternalInput")
    skt = nc.dram_tensor("skt", [SL, 128, SKT_W], BF16, kind="ExternalInput")
    gkt = nc.dram_tensor("gkt", [128, SL * 128], BF16, kind="ExternalInput")
    lv = nc.dram_tensor("lv", [SL, 128, LV_C * 65], BF16, kind="ExternalInput")
    sv = nc.dram_tensor("sv", [SL, 128, SV_C * 4 * 65], BF16, kind="ExternalInput")
    gv = nc.dram_tensor("gv", [128, SL * 65], BF16, kind="ExternalInput")
    # output: one contiguous 133KB DMA per PAIR-GROUP (2 pairs = 4 blocks)
    # of unnormalized [q, ctx|Z per block]; host divides + transposes
    # (PSUM is not DMA-able, so the DVE bounces pv into half of a
    # double-wide ob buffer first)
    o = nc.dram_tensor("o", [SL, PPS // 2, 128, 260], F32, kind="ExternalOutput")

    EXP = mybir.ActivationFunctionType.Exp

    with ExitStack() as es:
        ec = es.enter_context
        # double-buffered inputs (slot parity)
        qt_t = [ec(nc.sbuf_tensor(f"qt_t{i}", [128, T], BF16)) for i in range(2)]
        lkt_t = [ec(nc.sbuf_tensor(f"lkt_t{i}", [128, LKT_W], BF16)) for i in range(2)]
        skt_t = [ec(nc.sbuf_tensor(f"skt_t{i}", [128, SKT_W], BF16)) for i in range(2)]
        lv_t = [ec(nc.sbuf_tensor(f"lv_t{i}", [128, LV_C * 65], BF16)) for i in range(2)]
        sv_t = [ec(nc.sbuf_tensor(f"sv_t{i}", [128, SV_C * 4 * 65], BF16)) for i in range(2)]
        # globals are tiny: all slots resident, loaded once with one DMA each
        gkt_t = ec(nc.sbuf_tensor("gkt_t", [128, SL * 128], BF16))
        gv_t = ec(nc.sbuf_tensor("gv_t", [128, SL * 65], BF16))
        # per-pair working set
        psS = [ec(nc.psum_tensor(f"psS{i}", [128, 1536], F32)) for i in range(2)]  # 3 banks
        pv = [ec(nc.psum_tensor(f"pv{i}", [128, 512], F32)) for i in range(2)]     # 1 bank
        pp = [ec(nc.sbuf_tensor(f"pp{i}", [128, 1536], BF16)) for i in range(4)]
        warm = ec(nc.sbuf_tensor("warm", [128, 1], F32))
        ob = [ec(nc.sbuf_tensor(f"ob{i}", [128, 260], F32)) for i in range(2)]

        di = [[ec(nc.semaphore(f"di{i}{a}")) for a in range(2)] for i in range(2)]  # input loads, (parity, alternation)
        dg = ec(nc.semaphore("dg"))      # global k/v loads
        st = [ec(nc.semaphore(f"st{i}")) for i in range(2)]  # out stores, group%2 (matches ob buffers)
        pe_s = ec(nc.semaphore("pe_s"))  # +2 per pair: score banks01 / bank2 done
        pe_v = ec(nc.semaphore("pe_v"))  # +1 per pair: PV matmuls done
        act = ec(nc.semaphore("act"))    # +1 per pair: ACT exp done
        sch = ec(nc.semaphore("sch"))    # +1 per pair: DVE exp done
        dve = ec(nc.semaphore("dve"))    # +1 per pair: pv->ob copy done
        block = ec(nc.Block())

        # last waited-on cumulative value per di semaphore: a later phase
        # crossing that value must itself wait on it (race-checker rule),
        # which is free since the previous same-sem phase finished long ago
        chain = {}

        def phase_dmas(s, k):
            u = s % 2
            (q0, q1), (l0, l1), (s0, s1), (v0, v1), (c0, c1), _ = _slot_phases(s)[k]
            return [
                (qt_t[u][:, q0:q1], qt[s, :, q0:q1]),
                (lkt_t[u][:, l0:l1], lkt[s, :, l0:l1]),
                (skt_t[u][:, s0:s1], skt[s, :, s0:s1]),
                (lv_t[u][:, v0 * 65 : v1 * 65], lv[s, :, v0 * 65 : v1 * 65]),
                (sv_t[u][:, c0 * 260 : c1 * 260], sv[s, :, c0 * 260 : c1 * 260]),
            ]

        def phase_pieces(eng, s, k, wait=None):
            u = s % 2
            _, alt = DI_SEM[(s, k)]
            prev = chain.get((u, alt))
            for j, (dst, src) in enumerate(phase_dmas(s, k)):
                if wait is not None:
                    eng.wait_ge(pe_v, max(wait + 2 * j, 1))
                if j == 0 and prev is not None:
                    eng.wait_ge(di[u][alt], prev)
                eng.dma_start(dst, src).then_inc(di[u][alt], 16)
            chain[(u, alt)] = DI_GATE[(s, _slot_phases(s)[k][5])][2]

        @block.sync
        def _(sync):
            # sync queue: startup-critical loads, then ONLY stores, so a
            # store waiting on pe_v at the FIFO head never delays inputs
            phase_pieces(sync, 0, 0)
            sync.dma_start(gkt_t[:], gkt[:]).then_inc(dg, 16)
            for p in range(1, NP, 2):
                s, hb = divmod(p, PPS)
                g = p // 2
                sync.dma_start(
                    o[s, hb // 2, :, :], ob[g % 2][:, 0:260]
                ).wait_op(dve, p + 1, GE).then_inc(st[g % 2], 16)
            for i in range(2):
                sync.wait_ge(st[i], 16 * (NP // 4))

        @block.gpsimd
        def _(gpsimd):
            # gv first, with no wait: warms the SWDGE path during the
            # preamble (covers any one-time Q7 setup cost before the
            # latency-critical loads behind it)
            nc.gpsimd.dma_start(gv_t[:], gv[:]).then_inc(dg, 16)
            # don't let slot 0's later phases steal HBM bandwidth from the
            # startup-critical phase 0 on the sync queue
            gpsimd.wait_ge(di[0][0], 80)
            for k in range(1, 4):
                phase_pieces(gpsimd, 0, k)
            # whole-slot loads for slots 1-3, paced by pe_v so each slot's
            # buffers are free (previous occupant's pairs done) and the 5
            # transfers spread instead of bursting
            for s in range(1, SL):
                phase_pieces(gpsimd, s, 0, wait=16 * (s - 1))

        def emit_scores(p):
            s, hb = divmod(p, PPS)
            b = 2 * hb
            u = p % 2
            su = s % 2
            if p >= 2:
                # psS[u] WAR: with the grouped PE order (S,S,PV,PV) the
                # old PV-transitive coverage no longer holds - scores(p)
                # may be emitted before PV(p-2), so gate explicitly on
                # both halves of exp(p-2) having consumed psS[u]
                nc.tensor.wait_ge(act, p - 1)
                nc.tensor.wait_ge(sch, p - 1)
            if (s, hb) in DI_GATE:
                gu, galt, gval = DI_GATE[(s, hb)]
                nc.tensor.wait_ge(di[gu][galt], gval)
            qA = qt_t[su][:, b * B : (b + 1) * B]
            qB = qt_t[su][:, (b + 1) * B : (b + 2) * B]
            qAB = qt_t[su][:, b * B : (b + 2) * B]
            w1a, w2a = 32 * b, 32 * b + 224
            w1b, w2b = w1a + 32, w2a + 32
            # banks 0-1 (ACT exp) first, bank 2 (DVE exp) last; pe_s +1 at
            # each boundary
            mms = (
                (C_SP1A, 128, skt_t[su][:, w1a : w1a + 128], qA),
                (C_SP1B, 128, skt_t[su][:, w1b : w1b + 128], qB),
                (C_SP2A, 128, skt_t[su][:, w2a : w2a + 128], qA),
                (C_SP2B, 128, skt_t[su][:, w2b : w2b + 128], qB),
                (C_G, 256, gkt_t[:, s * 128 : (s + 1) * 128], qAB),
                (C_LOC1, 256, lkt_t[su][:, (b + 1) * B : (b + 2) * B], qAB),
                (C_LOC2, 256, lkt_t[su][:, (b + 2) * B : (b + 3) * B], qAB),
                (C_LOC0, 128, lkt_t[su][:, b * B : (b + 1) * B], qA),
                (C_LOC3, 128, lkt_t[su][:, (b + 3) * B : (b + 4) * B], qB),
            )
            for kk, (col, w, lhsT, rhs) in enumerate(mms):
                if p == 0 and kk == 4:
                    nc.tensor.wait_ge(dg, 32)  # globals loaded (covers gv too)
                mm = nc.tensor.matmul(
                    psS[u][:, col : col + w],
                    lhsT, rhs,
                    start=True, stop=True,
                )
                if kk in (5, 8):
                    mm.then_inc(pe_s, 1)

        def emit_pv(p):
            s, hb = divmod(p, PPS)
            b = 2 * hb
            u = p % 2
            su = s % 2
            j3 = p % 4
            if p >= 2:
                nc.tensor.wait_ge(dve, p - 1)  # pv[u] free (copy p-2 done)
            kk = 0
            for blk in range(2):
                bb = b + blk
                w1, w2 = 32 * bb, 32 * bb + 224
                c1, r1 = divmod(w1, 128)
                c2, r2 = divmod(w2, 128)
                p1, p2 = r1 // 32, r2 // 32
                if blk == 0:
                    lhs = (C_SP1A, C_SP2A, C_G, C_LOC1, C_LOC2, C_LOC0)
                    lvs = (bb + 1, bb + 2, bb)
                else:
                    lhs = (C_SP1B, C_SP2B, C_G + 128, C_LOC1 + 128,
                           C_LOC2 + 128, C_LOC3)
                    lvs = (bb, bb + 1, bb + 2)
                rhss = (
                    sv_t[su][:, (c1 * 4 + p1) * 65 : (c1 * 4 + p1) * 65 + 65],
                    sv_t[su][:, (c2 * 4 + p2) * 65 : (c2 * 4 + p2) * 65 + 65],
                    gv_t[:, s * 65 : (s + 1) * 65],
                    lv_t[su][:, lvs[0] * 65 : lvs[0] * 65 + 65],
                    lv_t[su][:, lvs[1] * 65 : lvs[1] * 65 + 65],
                    lv_t[su][:, lvs[2] * 65 : lvs[2] * 65 + 65],
                )
                out = pv[u][:, blk * 65 : blk * 65 + 65]
                for j in range(6):
                    mm = nc.tensor.matmul(
                        out, pp[j3][:, lhs[j] : lhs[j] + 128], rhss[j],
                        start=(j == 0), stop=(j == 5),
                    )
                    if kk == 0:
                        mm.wait_op(act, p + 1, GE)  # pp ACT half ready
                    elif kk == 4:
                        mm.wait_op(sch, p + 1, GE)  # pp DVE half ready
                    if kk == 11:
                        mm.then_inc(pe_v, 1)
                    kk += 1

        @block.tensor
        def _(tensor):
            # warm the HAM activity monitor during the input-load dead
            # time so the first pairs run at 2.4 GHz: ~4.3us of dummy
            # matmuls on (uninitialized, never-DMA'd) SBUF junk; psS is
            # reset by scores(0)'s start=True writes
            for _ in range(10):
                nc.tensor.matmul(
                    psS[0][:, 0:512], pp[0][:, 0:128], pp[0][:, 0:512],
                    start=True, stop=True,
                )
            # grouped order [S(2k), S(2k+1), PV(2k-2), PV(2k-1)]: the DVE
            # gets a full extra pair of slack for its exp+copy stream, and
            # consecutive ACT exps queue back-to-back (the banks01 of the
            # group's second S complete before the first exp finishes)
            emit_scores(0)
            emit_scores(1)
            for k in range(1, NP // 2):
                emit_scores(2 * k)
                emit_scores(2 * k + 1)
                emit_pv(2 * k - 2)
                emit_pv(2 * k - 1)
            emit_pv(NP - 2)
            emit_pv(NP - 1)

        @block.scalar
        def _(scalar):
            # touch Exp once so the ACT table load overlaps the input DMA head
            nc.scalar.activation(warm[:], warm[:], EXP, scale=0.0)
            for p in range(NP):
                u = p % 2
                j3 = p % 4
                # pp[j3] WAR vs PV(p-4) is transitively covered: exp(p)
                # waits on scores(p)'s mid-increment, and scores(p) follow
                # PV(p-4) in PE program order (grouped schedule).
                nc.scalar.activation(
                    pp[j3][:, 0:1024], psS[u][:, 0:1024], EXP, scale=0.125
                ).wait_op(pe_s, 2 * p + 1, GE).then_inc(act, 1)

        def emit_copy(vector, q):
            # copy pair q's pv into its ob half; lags the exp stream by one
            # pair so TS(p+1) is never queued behind a copy that waits on
            # PV(q) (that program-order loop was an earlier period limiter)
            g = q // 2
            if q % 2 == 0 and g >= 2:
                # ob[g%2] free once store(g-2) completed
                vector.wait_ge(st[g % 2], 16 * (g // 2))
            nc.vector.tensor_copy(
                ob[g % 2][:, (q % 2) * 130 : (q % 2) * 130 + 130],
                pv[q % 2][:, 0:130],
            ).wait_op(pe_v, q + 1, GE).then_inc(dve, 1)

        @block.vector
        def _(vector):
            for p in range(NP):
                # Schraudolph exp of bank 2; pp[p%4] WAR vs PV(p-4) covered
                # like the ACT half (waits scores(p) done via pe_s).
                nc.vector.tensor_scalar(
                    pp[p % 4][:, 1024:1536].bitcast(mybir.dt.int16),
                    psS[p % 2][:, 1024:1536],
                    SCH_A, SCH_B,
                    mybir.AluOpType.mult, mybir.AluOpType.add,
                ).wait_op(pe_s, 2 * p + 2, GE).then_inc(sch, 1)
                if p >= 1:
                    emit_copy(vector, p - 1)
            emit_copy(vector, NP - 1)

    return nc


def _prepare(inputs):
    import ml_dtypes

    bf = ml_dtypes.bfloat16
    f = np.float32
    q = np.asarray(inputs["query_layer"], f).reshape(NH, T, D)
    k = np.asarray(inputs["key_layer"], f).reshape(NH, T, D)
    v = np.asarray(inputs["value_layer"], f).reshape(NH, T, D)
    sk = np.asarray(inputs["sparse_key"], f).reshape(NH, TSP, D)
    svv = np.asarray(inputs["sparse_value"], f).reshape(NH, TSP, D)
    gk = np.asarray(inputs["global_key"], f).reshape(NH, G, D)
    gvv = np.asarray(inputs["global_value"], f).reshape(NH, G, D)
    am = np.repeat(np.asarray(inputs["attention_mask"], f)[:, 0, 0, :], H, 0)
    sm = np.repeat(np.asarray(inputs["sparse_mask"], f)[:, 0, 0, :], H, 0)
    gm = np.repeat(np.asarray(inputs["global_mask"], f)[:, 0, 0, :], H, 0)

    # all matmul operands are zero-padded to 128 rows on the host so the
    # PE always sees uniform, junk-free 128-row tiles
    qt = np.zeros((NH, 128, T), f)
    qt[:, :64] = q.transpose(0, 2, 1)
    qt = qt.astype(bf)

    lkt = np.zeros((NH, 128, LKT_W), f)
    lkt[:, :64, B : B + T] = k.transpose(0, 2, 1)
    lkt = lkt.astype(bf)

    skt = np.zeros((NH, 128, SKT_W), f)
    skt[:, :64, 160 : 160 + TSP] = sk.transpose(0, 2, 1)
    skt = skt.astype(bf)

    gkt = np.zeros((NH, 128, 128), f)
    gkt[:, :64, :G] = gk.transpose(0, 2, 1)
    gkt = gkt.astype(bf)
    # per-core: [128, SL*128] (slot-minor so one DMA loads all slots)
    gktj = np.ascontiguousarray(
        gkt.reshape(NCORES, SL, 128, 128).transpose(0, 2, 1, 3)
    ).reshape(NCORES, 128, SL * 128)

    # V_aug rows scaled by exp(mask); pad rows are all-zero
    em_l = np.zeros((NH, LKT_W), f)
    em_l[:, B : B + T] = np.exp(am)
    lvp = np.zeros((NH, LKT_W, 65), f)
    lvp[:, B : B + T, :64] = v
    lvp[:, :, 64] = 1.0
    lvp *= em_l[:, :, None]
    lvp = np.ascontiguousarray(
        lvp.reshape(NH, LV_C, 128, 65).transpose(0, 2, 1, 3)
    ).reshape(NH, 128, LV_C * 65).astype(bf)

    SVP_W = 96 + SV_C * 128
    em_s = np.zeros((NH, SVP_W), f)
    em_s[:, 160 : 160 + TSP] = np.exp(sm)
    sv_pad = np.zeros((NH, SVP_W, 65), f)
    sv_pad[:, 160 : 160 + TSP, :64] = svv
    sv_pad[:, :, 64] = 1.0
    sv_pad *= em_s[:, :, None]
    svp = np.empty((NH, 4, 128, SV_C, 65), f)
    for p in range(4):
        svp[:, p] = (
            sv_pad[:, 32 * p : 32 * p + SV_C * 128]
            .reshape(NH, SV_C, 128, 65)
            .transpose(0, 2, 1, 3)
        )
    # [NH, 4, 128, SV_C, 65] -> [NH, 128, SV_C, 4, 65] (chunk-major so
    # chunk-range load pieces are contiguous per partition)
    svp = np.ascontiguousarray(svp.transpose(0, 2, 3, 1, 4)).astype(bf)
    svp = svp.reshape(NH, 128, SV_C * 4 * 65)

    gvp = np.zeros((NH, 128, 65), f)
    gvp[:, :G, :64] = gvv
    gvp[:, :G, 64] = 1.0
    gvp[:, :G] *= np.exp(gm)[:, :, None]
    gvp = gvp.astype(bf)
    gvj = np.ascontiguousarray(
        gvp.reshape(NCORES, SL, 128, 65).transpose(0, 2, 1, 3)
    ).reshape(NCORES, 128, SL * 65)

    return [
        {
            "qt": qt[c * SL : (c + 1) * SL],
            "lkt": lkt[c * SL : (c + 1) * SL],
            "skt": skt[c * SL : (c + 1) * SL],
            "gkt": gktj[c],
            "lv": lvp[c * SL : (c + 1) * SL],
            "sv": svp[c * SL : (c + 1) * SL],
            "gv": gvj[c],
        }
        for c in range(NCORES)
    ]


_NC_CACHE = {}
LAST_RESULTS = None


def kernel(**inputs):
    global LAST_RESULTS
    if "nc" not in _NC_CACHE:
        _NC_CACHE["nc"] = _build_bass()
    nc = _NC_CACHE["nc"]
    in_maps = _prepare(inputs)
    res = run_bass_kernel_spmd(nc, in_maps, core_ids=list(range(NCORES)))
    LAST_RESULTS = res
    out = np.empty((NH, T, D), np.float32)
    for c in range(NCORES):
        # o is [SL, PPS, 128 q, ctxA|ZA|ctxB|ZB]; normalize + reorder to
        # [SL, T, D] on the host
        ot = res.results[c]["o"]
        ab = ot.reshape(SL, PPS // 2, 128, 4, 65).transpose(0, 1, 3, 2, 4)
        ab = ab.reshape(SL, T, 65)
        out[c * SL : (c + 1) * SL] = ab[:, :, 0:64] / ab[:, :, 64:65]
    return out.reshape(N, H, T, D)


# revision 39
# speedup vs baseline: 1.1117x; 1.1117x over previous
"""Block-local sparse attention (LSG-style) on 8 TRN2 NeuronCores.

Sharding: the 32 (n, h) pairs are split 4-per-core (data/head parallel, no
collectives). Host-side numpy prep re-lays-out the inputs so the device
kernel needs no transposes, all bf16:

  - qt : Q^T per head, zero-padded to [128, T] on the host.
  - lkt/skt/gkt: local/sparse/global K^T, token-padded with zeros and
    row-padded to 128 partitions (uniform 128-row PE tile shapes keep the
    HAM activity monitor from down-clocking the tensor engine).
  - lv/sv/gv: V with a ones column appended (col 64), chunked [128, c, 65],
    every row scaled by exp(mask): softmax(QK/8 + m) @ V is computed as
    sum_t exp(s_t) e^{m_t} [V_t, 1]; the divide by the accumulated last
    column happens ON THE HOST (output is unnormalized ctx|Z), so the DVE
    has no normalization work on device.
    sv holds 4 phase-shifted copies so the 32-token-granular sparse windows
    always start at partition 0.

The device processes query-block PAIRS: 9 score matmuls per pair into a
3-bank PSUM region [128, 1536].  exp is split across two engines at PSUM
BANK granularity (an engine reading a PSUM bank while another engine
touches the same bank hard-crashes the device, so the split must be
bank-aligned):
  - ACT: one activation exp(S/8) over cols 0:1024 (banks 0-1: sparse,
    global, local b+1) -> bf16 pp.  ~1.0-1.1 us/pair; the limiter.
  - DVE: Schraudolph exp over cols 1024:1536 (bank 2: local b+2, b, b+3)
    via one tensor_scalar: bf16 bits of exp(x/8) ~= int16(x*A + B)
    (rel err ~2%/element on ~36% of context tokens; end-to-end rel err
    ~1.1e-2 vs the 2e-2 gate).  ~0.68 us/pair.
Then 12 PV matmuls (6 per block, N=65) accumulate [q, V|Z] into pv cols
0:65 / 65:130, and ONE 66.5KB DMA per pair stores pv straight from PSUM
(PV(p+2) is gated on store(p) completion so the PE never writes a bank a
store is still reading).

DMA queues: each queue costs a flat ~0.6us per DMA instruction
(descriptor generation), so queue ASSIGNMENT matters more than bytes:
  - sync/HWDGE queue: the startup-critical phase-0 + gkt, then ONLY the
    64 pair stores (a store waiting at the FIFO head can therefore never
    delay input loads - that head-of-line blocking caused slot-boundary
    PE starvation and HAM re-throttles in earlier versions).
  - gpsimd/SWDGE queue: everything else - gv (first, to swallow any
    one-time SWDGE setup), the rest of slot 0 in 3 column-phases, then
    slots 1-3 as 5 whole-slot transfers each, paced by pe_v.

Known hardware landmine (cost several device wedges in a previous
session): concurrent same-PSUM-bank access by two engines (ACT read + DVE
read, or PE matmul write + DVE read) hard-crashes the device
(NRT_EXEC_UNIT_UNRECOVERABLE).  The bank-aligned ACT/DVE split plus the
act/sch/st gates keep every bank single-toucher while in use.
"""

from contextlib import ExitStack

import numpy as np

import concourse.bass as bass
import concourse.mybir as mybir
from concourse.bass_utils import run_bass_kernel_spmd

N, H, T, D = 2, 16, 4096, 64
B = 128          # query block
NB = T // B      # 32
G = 64           # global tokens
TSP = T // 4     # sparse tokens (1024)
NH = N * H       # 32
NCORES = 8
SL = NH // NCORES  # 4 heads per core
NP = SL * NB // 2  # 64 block-pairs per core
PPS = NB // 2      # 16 pairs per slot

LKT_W = T + 2 * B            # 4352 padded local tokens
SKT_W = TSP + 320            # 1344 padded sparse tokens
LV_C = LKT_W // 128          # 34 local V chunks
SV_C = 11                    # sparse V chunks per phase

F32 = mybir.dt.float32
BF16 = mybir.dt.bfloat16
GE = "sem-ge"

# column layout of the per-pair score/prob tile [128, 1536] (3 PSUM banks;
# regions never cross a 512-col bank boundary).  Cols 0:1024 (banks 0-1)
# are exp'd by ACT; cols 1024:1536 (bank 2) by the DVE Schraudolph trick.
C_SP1A, C_SP1B = 0, 128
C_SP2A, C_SP2B = 256, 384
C_G = 512        # 256 wide: q of both blocks
C_LOC1 = 768     # 256 wide: local chunk b+1, both blocks
C_LOC2 = 1024    # 256 wide: local chunk b+2, both blocks (DVE exp)
C_LOC0 = 1280    # 128: local chunk b, block A only (DVE exp)
C_LOC3 = 1408    # 128: local chunk b+3, block B only (DVE exp)

# Schraudolph constants: bf16 bits of exp(x*0.125) ~= int16(x*SCH_A + SCH_B)
SCH_A = float(128 * 1.4426950408889634 * 0.125)
SCH_B = 16256.0 - 0.057 * 128.0

# Input staging.  Slot 0 loads in column-phases (phase k0a gates
# scores(0) on just qt/lkt/skt; k0b gates PV(0) on lv/sv); slots 1-3
# load each tensor with ONE whole-slot DMA.  Ranges: qt/lkt/skt are
# column ranges, lv/sv are chunk ranges.  gate_hb is the first pair
# (within the slot) needing the phase; 'pv0' gates emit_pv(0).
PHASES_S0 = (
    dict(qt=(0, 512), lkt=(0, 768), skt=(0, 448), gate_hb=0),
    dict(lv=(0, 8), sv=(0, 3), gate_hb="pv0"),
    dict(qt=(512, 1024), lkt=(768, 1280), skt=(448, 576), lv=(8, 12),
         sv=(3, 4), gate_hb=2),
    dict(qt=(1024, 2560), lkt=(1280, 2816), skt=(576, 960), lv=(12, 24),
         sv=(4, 7), gate_hb=4),
    dict(qt=(2560, 4096), lkt=(2816, LKT_W), skt=(960, SKT_W),
         lv=(24, LV_C), sv=(7, SV_C), gate_hb=10),
)
PHASE_FULL = dict(qt=(0, T), lkt=(0, LKT_W), skt=(0, SKT_W),
                  lv=(0, LV_C), sv=(0, SV_C), gate_hb=0)


def _slot_phases(s):
    return PHASES_S0 if s == 0 else (PHASE_FULL,)


# A semaphore wait is only race-free at the end of a maximal run of
# consecutive instructions updating that semaphore, so consecutive phases
# alternate between two semaphores per slot parity: di[parity][phase_seq%2].
# DI_SEM[(s, k)] = (parity, alt) and DI_GATE[(s, hb)] = (parity, alt, value).
DI_SEM = {}
DI_GATE = {}
for _u in range(2):
    _cum = [0, 0]
    _seq = 0
    for _s in range(_u, SL, 2):
        for _k, _ph in enumerate(_slot_phases(_s)):
            _alt = _seq % 2
            _ndma = sum(1 for _key in ("qt", "lkt", "skt", "lv", "sv")
                        if _key in _ph)
            _cum[_alt] += 16 * _ndma
            DI_SEM[(_s, _k)] = (_u, _alt)
            DI_GATE[(_s, _ph["gate_hb"])] = (_u, _alt, _cum[_alt])
            _seq += 1


def _build_bass():
    nc = bass.Bass("TRN2", num_devices=NCORES, debug=False)

    qt = nc.dram_tensor("qt", [SL, 128, T], BF16, kind="ExternalInput")
    lkt = nc.dram_tensor("lkt", [SL, 128, LKT_W], BF16, kind="ExternalInput")
    skt = nc.dram_tensor("skt", [SL, 128, SKT_W], BF16, kind="ExternalInput")
    gkt = nc.dram_tensor("gkt", [128, SL * 128], BF16, kind="ExternalInput")
    lv = nc.dram_tensor("lv", [SL, 128, LV_C * 65], BF16, kind="ExternalInput")
    sv = nc.dram_tensor("sv", [SL, 128, SV_C * 4 * 65], BF16, kind="ExternalInput")
    gv = nc.dram_tensor("gv", [128, SL * 65], BF16, kind="ExternalInput")
    # output: one contiguous 133KB DMA per PAIR-GROUP (2 pairs = 4 blocks)
    # of unnormalized [q, ctx|Z per block]; host divides + transposes
    # (PSUM is not DMA-able, so the DVE bounces pv into half of a
    # double-wide ob buffer first)
    o = nc.dram_tensor("o", [SL, PPS // 2, 128, 260], F32, kind="ExternalOutput")

    EXP = mybir.ActivationFunctionType.Exp

    with ExitStack() as es:
        ec = es.enter_context
        # double-buffered inputs (slot parity)
        qt_t = [ec(nc.sbuf_tensor(f"qt_t{i}", [128, T], BF16)) for i in range(2)]
        lkt_t = [ec(nc.sbuf_tensor(f"lkt_t{i}", [128, LKT_W], BF16)) for i in range(2)]
        skt_t = [ec(nc.sbuf_tensor(f"skt_t{i}", [128, SKT_W], BF16)) for i in range(2)]
        lv_t = [ec(nc.sbuf_tensor(f"lv_t{i}", [128, LV_C * 65], BF16)) for i in range(2)]
        sv_t = [ec(nc.sbuf_tensor(f"sv_t{i}", [128, SV_C * 4 * 65], BF16)) for i in range(2)]
        # globals are tiny: all slots resident, loaded once with one DMA each
        gkt_t = ec(nc.sbuf_tensor("gkt_t", [128, SL * 128], BF16))
        gv_t = ec(nc.sbuf_tensor("gv_t", [128, SL * 65], BF16))
        # per-pair working set
        psS = [ec(nc.psum_tensor(f"psS{i}", [128, 1536], F32)) for i in range(2)]  # 3 banks
        pv = [ec(nc.psum_tensor(f"pv{i}", [128, 512], F32)) for i in range(2)]     # 1 bank
        pp = [ec(nc.sbuf_tensor(f"pp{i}", [128, 1536], BF16)) for i in range(4)]
        warm = ec(nc.sbuf_tensor("warm", [128, 1], F32))
        ob = [ec(nc.sbuf_tensor(f"ob{i}", [128, 260], F32)) for i in range(2)]

        di = [[ec(nc.semaphore(f"di{i}{a}")) for a in range(2)] for i in range(2)]  # input loads, (parity, alternation)
        dg = ec(nc.semaphore("dg"))      # global k/v loads
        st = [ec(nc.semaphore(f"st{i}")) for i in range(2)]  # out stores, group%2 (matches ob buffers)
        pe_s = ec(nc.semaphore("pe_s"))  # +2 per pair: score banks01 / bank2 done
        pe_v = ec(nc.semaphore("pe_v"))  # +1 per pair: PV matmuls done
        act = ec(nc.semaphore("act"))    # +1 per pair: ACT exp done
        sch = ec(nc.semaphore("sch"))    # +1 per pair: DVE exp done
        dve = ec(nc.semaphore("dve"))    # +1 per pair: pv->ob copy done
        block = ec(nc.Block())

        # last waited-on cumulative value per di semaphore: a later phase
        # crossing that value must itself wait on it (race-checker rule),
        # which is free since the previous same-sem phase finished long ago
        chain = {}

        def phase_dmas(s, k):
            u = s % 2
            ph = _slot_phases(s)[k]
            out = []
            if "qt" in ph:
                q0, q1 = ph["qt"]
                out.append((qt_t[u][:, q0:q1], qt[s, :, q0:q1]))
            if "lkt" in ph:
                l0, l1 = ph["lkt"]
                out.append((lkt_t[u][:, l0:l1], lkt[s, :, l0:l1]))
            if "skt" in ph:
                s0, s1 = ph["skt"]
                out.append((skt_t[u][:, s0:s1], skt[s, :, s0:s1]))
            if "lv" in ph:
                v0, v1 = ph["lv"]
                out.append((lv_t[u][:, v0 * 65 : v1 * 65],
                            lv[s, :, v0 * 65 : v1 * 65]))
            if "sv" in ph:
                c0, c1 = ph["sv"]
                out.append((sv_t[u][:, c0 * 260 : c1 * 260],
                            sv[s, :, c0 * 260 : c1 * 260]))
            return out

        def phase_pieces(eng, s, k, wait=None):
            u = s % 2
            _, alt = DI_SEM[(s, k)]
            prev = chain.get((u, alt))
            for j, (dst, src) in enumerate(phase_dmas(s, k)):
                if j == 0:
                    if wait is not None:
                        eng.wait_ge(pe_v, wait)
                    if prev is not None:
                        eng.wait_ge(di[u][alt], prev)
                eng.dma_start(dst, src).then_inc(di[u][alt], 16)
            chain[(u, alt)] = DI_GATE[(s, _slot_phases(s)[k]["gate_hb"])][2]

        @block.sync
        def _(sync):
            # sync queue: startup-critical loads, then ONLY stores, so a
            # store waiting on pe_v at the FIFO head never delays inputs
            phase_pieces(sync, 0, 0)
            sync.dma_start(gkt_t[:], gkt[:]).then_inc(dg, 16)
            for p in range(1, NP, 2):
                s, hb = divmod(p, PPS)
                g = p // 2
                sync.dma_start(
                    o[s, hb // 2, :, :], ob[g % 2][:, 0:260]
                ).wait_op(dve, p + 1, GE).then_inc(st[g % 2], 16)
            for i in range(2):
                sync.wait_ge(st[i], 16 * (NP // 4))

        @block.gpsimd
        def _(gpsimd):
            # gv first, with no wait: warms the SWDGE path during the
            # preamble (covers any one-time Q7 setup cost before the
            # latency-critical loads behind it)
            nc.gpsimd.dma_start(gv_t[:], gv[:]).then_inc(dg, 16)
            # k0b (lv/sv for the first pairs) runs concurrently with the
            # sync queue's k0a
            phase_pieces(gpsimd, 0, 1)
            # (the chain wait on the next phase already serializes it
            # behind k0a, giving the startup-critical sync loads priority)
            for k in range(2, len(PHASES_S0)):
                phase_pieces(gpsimd, 0, k)
            # whole-slot input loads for slots 1-3: issue as early as the
            # buffer-free condition allows (pe_v wait on the first DMA
            # only - pacing beyond that creates a late-loads -> stalled
            # pairs -> later pe_v feedback loop)
            for s in range(1, SL):
                phase_pieces(gpsimd, s, 0,
                             wait=16 * (s - 1) if s >= 2 else None)

        def emit_scores(p):
            s, hb = divmod(p, PPS)
            b = 2 * hb
            u = p % 2
            su = s % 2
            if (s, hb) in DI_GATE:
                gu, galt, gval = DI_GATE[(s, hb)]
                nc.tensor.wait_ge(di[gu][galt], gval)
            qA = qt_t[su][:, b * B : (b + 1) * B]
            qB = qt_t[su][:, (b + 1) * B : (b + 2) * B]
            qAB = qt_t[su][:, b * B : (b + 2) * B]
            w1a, w2a = 32 * b, 32 * b + 224
            w1b, w2b = w1a + 32, w2a + 32
            # banks 0-1 (ACT exp) first, bank 2 (DVE exp) last; pe_s +1 at
            # each boundary
            mms = (
                (C_SP1A, 128, skt_t[su][:, w1a : w1a + 128], qA),
                (C_SP1B, 128, skt_t[su][:, w1b : w1b + 128], qB),
                (C_SP2A, 128, skt_t[su][:, w2a : w2a + 128], qA),
                (C_SP2B, 128, skt_t[su][:, w2b : w2b + 128], qB),
                (C_G, 256, gkt_t[:, s * 128 : (s + 1) * 128], qAB),
                (C_LOC1, 256, lkt_t[su][:, (b + 1) * B : (b + 2) * B], qAB),
                (C_LOC2, 256, lkt_t[su][:, (b + 2) * B : (b + 3) * B], qAB),
                (C_LOC0, 128, lkt_t[su][:, b * B : (b + 1) * B], qA),
                (C_LOC3, 128, lkt_t[su][:, (b + 3) * B : (b + 4) * B], qB),
            )
            for kk, (col, w, lhsT, rhs) in enumerate(mms):
                if p == 0 and kk == 4:
                    nc.tensor.wait_ge(dg, 32)  # globals loaded (covers gv too)
                mm = nc.tensor.matmul(
                    psS[u][:, col : col + w],
                    lhsT, rhs,
                    start=True, stop=True,
                )
                if kk in (5, 8):
                    mm.then_inc(pe_s, 1)

        def emit_pv(p):
            s, hb = divmod(p, PPS)
            b = 2 * hb
            u = p % 2
            su = s % 2
            j3 = p % 4
            if p == 0:
                # lv/sv of the first pairs arrive via phase k0b
                gu, galt, gval = DI_GATE[(0, "pv0")]
                nc.tensor.wait_ge(di[gu][galt], gval)
            if p >= 2:
                nc.tensor.wait_ge(dve, p - 1)  # pv[u] free (copy p-2 done)
            kk = 0
            for blk in range(2):
                bb = b + blk
                w1, w2 = 32 * bb, 32 * bb + 224
                c1, r1 = divmod(w1, 128)
                c2, r2 = divmod(w2, 128)
                p1, p2 = r1 // 32, r2 // 32
                if blk == 0:
                    lhs = (C_SP1A, C_SP2A, C_G, C_LOC1, C_LOC2, C_LOC0)
                    lvs = (bb + 1, bb + 2, bb)
                else:
                    lhs = (C_SP1B, C_SP2B, C_G + 128, C_LOC1 + 128,
                           C_LOC2 + 128, C_LOC3)
                    lvs = (bb, bb + 1, bb + 2)
                rhss = (
                    sv_t[su][:, (c1 * 4 + p1) * 65 : (c1 * 4 + p1) * 65 + 65],
                    sv_t[su][:, (c2 * 4 + p2) * 65 : (c2 * 4 + p2) * 65 + 65],
                    gv_t[:, s * 65 : (s + 1) * 65],
                    lv_t[su][:, lvs[0] * 65 : lvs[0] * 65 + 65],
                    lv_t[su][:, lvs[1] * 65 : lvs[1] * 65 + 65],
                    lv_t[su][:, lvs[2] * 65 : lvs[2] * 65 + 65],
                )
                out = pv[u][:, blk * 65 : blk * 65 + 65]
                for j in range(6):
                    mm = nc.tensor.matmul(
                        out, pp[j3][:, lhs[j] : lhs[j] + 128], rhss[j],
                        start=(j == 0), stop=(j == 5),
                    )
                    if kk == 0:
                        mm.wait_op(act, p + 1, GE)  # pp ACT half ready
                    elif kk == 4:
                        mm.wait_op(sch, p + 1, GE)  # pp DVE half ready
                    if kk == 11:
                        mm.then_inc(pe_v, 1)
                    kk += 1

        @block.tensor
        def _(tensor):
            # warm the HAM activity monitor during the input-load dead
            # time so the first pairs run at 2.4 GHz: ~4.3us of dummy
            # matmuls on (uninitialized, never-DMA'd) SBUF junk; psS is
            # reset by scores(0)'s start=True writes
            for _ in range(10):
                nc.tensor.matmul(
                    psS[0][:, 0:512], pp[0][:, 0:128], pp[0][:, 0:512],
                    start=True, stop=True,
                )
            emit_scores(0)
            emit_scores(1)
            for p in range(NP):
                emit_pv(p)
                if p + 2 < NP:
                    emit_scores(p + 2)

        @block.scalar
        def _(scalar):
            # touch Exp once so the ACT table load overlaps the input DMA head
            nc.scalar.activation(warm[:], warm[:], EXP, scale=0.0)
            for p in range(NP):
                u = p % 2
                j3 = p % 4
                # pp[j3] WAR vs PV(p-4) is transitively covered: exp(p)
                # waits on scores(p)'s mid-increment, and scores(p) follow
                # PV(p-4) in PE program order (grouped schedule).
                nc.scalar.activation(
                    pp[j3][:, 0:1024], psS[u][:, 0:1024], EXP, scale=0.125
                ).wait_op(pe_s, 2 * p + 1, GE).then_inc(act, 1)

        def emit_copy(vector, q):
            # copy pair q's pv into its ob half; lags the exp stream by one
            # pair so TS(p+1) is never queued behind a copy that waits on
            # PV(q) (that program-order loop was an earlier period limiter)
            g = q // 2
            if q % 2 == 0 and g >= 2:
                # ob[g%2] free once store(g-2) completed
                vector.wait_ge(st[g % 2], 16 * (g // 2))
            nc.vector.tensor_copy(
                ob[g % 2][:, (q % 2) * 130 : (q % 2) * 130 + 130],
                pv[q % 2][:, 0:130],
            ).wait_op(pe_v, q + 1, GE).then_inc(dve, 1)

        @block.vector
        def _(vector):
            for p in range(NP):
                # Schraudolph exp of bank 2; pp[p%4] WAR vs PV(p-4) covered
                # like the ACT half (waits scores(p) done via pe_s).
                nc.vector.tensor_scalar(
                    pp[p % 4][:, 1024:1536].bitcast(mybir.dt.int16),
                    psS[p % 2][:, 1024:1536],
                    SCH_A, SCH_B,
                    mybir.AluOpType.mult, mybir.AluOpType.add,
                ).wait_op(pe_s, 2 * p + 2, GE).then_inc(sch, 1)
                if p >= 1:
                    emit_copy(vector, p - 1)
            emit_copy(vector, NP - 1)

    return nc


def _prepare(inputs):
    import ml_dtypes

    bf = ml_dtypes.bfloat16
    f = np.float32
    q = np.asarray(inputs["query_layer"], f).reshape(NH, T, D)
    k = np.asarray(inputs["key_layer"], f).reshape(NH, T, D)
    v = np.asarray(inputs["value_layer"], f).reshape(NH, T, D)
    sk = np.asarray(inputs["sparse_key"], f).reshape(NH, TSP, D)
    svv = np.asarray(inputs["sparse_value"], f).reshape(NH, TSP, D)
    gk = np.asarray(inputs["global_key"], f).reshape(NH, G, D)
    gvv = np.asarray(inputs["global_value"], f).reshape(NH, G, D)
    am = np.repeat(np.asarray(inputs["attention_mask"], f)[:, 0, 0, :], H, 0)
    sm = np.repeat(np.asarray(inputs["sparse_mask"], f)[:, 0, 0, :], H, 0)
    gm = np.repeat(np.asarray(inputs["global_mask"], f)[:, 0, 0, :], H, 0)

    # all matmul operands are zero-padded to 128 rows on the host so the
    # PE always sees uniform, junk-free 128-row tiles
    qt = np.zeros((NH, 128, T), f)
    qt[:, :64] = q.transpose(0, 2, 1)
    qt = qt.astype(bf)

    lkt = np.zeros((NH, 128, LKT_W), f)
    lkt[:, :64, B : B + T] = k.transpose(0, 2, 1)
    lkt = lkt.astype(bf)

    skt = np.zeros((NH, 128, SKT_W), f)
    skt[:, :64, 160 : 160 + TSP] = sk.transpose(0, 2, 1)
    skt = skt.astype(bf)

    gkt = np.zeros((NH, 128, 128), f)
    gkt[:, :64, :G] = gk.transpose(0, 2, 1)
    gkt = gkt.astype(bf)
    # per-core: [128, SL*128] (slot-minor so one DMA loads all slots)
    gktj = np.ascontiguousarray(
        gkt.reshape(NCORES, SL, 128, 128).transpose(0, 2, 1, 3)
    ).reshape(NCORES, 128, SL * 128)

    # V_aug rows scaled by exp(mask); pad rows are all-zero
    em_l = np.zeros((NH, LKT_W), f)
    em_l[:, B : B + T] = np.exp(am)
    lvp = np.zeros((NH, LKT_W, 65), f)
    lvp[:, B : B + T, :64] = v
    lvp[:, :, 64] = 1.0
    lvp *= em_l[:, :, None]
    lvp = np.ascontiguousarray(
        lvp.reshape(NH, LV_C, 128, 65).transpose(0, 2, 1, 3)
    ).reshape(NH, 128, LV_C * 65).astype(bf)

    SVP_W = 96 + SV_C * 128
    em_s = np.zeros((NH, SVP_W), f)
    em_s[:, 160 : 160 + TSP] = np.exp(sm)
    sv_pad = np.zeros((NH, SVP_W, 65), f)
    sv_pad[:, 160 : 160 + TSP, :64] = svv
    sv_pad[:, :, 64] = 1.0
    sv_pad *= em_s[:, :, None]
    svp = np.empty((NH, 4, 128, SV_C, 65), f)
    for p in range(4):
        svp[:, p] = (
            sv_pad[:, 32 * p : 32 * p + SV_C * 128]
            .reshape(NH, SV_C, 128, 65)
            .transpose(0, 2, 1, 3)
        )
    # [NH, 4, 128, SV_C, 65] -> [NH, 128, SV_C, 4, 65] (chunk-major so
    # chunk-range load pieces are contiguous per partition)
    svp = np.ascontiguousarray(svp.transpose(0, 2, 3, 1, 4)).astype(bf)
    svp = svp.reshape(NH, 128, SV_C * 4 * 65)

    gvp = np.zeros((NH, 128, 65), f)
    gvp[:, :G, :64] = gvv
    gvp[:, :G, 64] = 1.0
    gvp[:, :G] *= np.exp(gm)[:, :, None]
    gvp = gvp.astype(bf)
    gvj = np.ascontiguousarray(
        gvp.reshape(NCORES, SL, 128, 65).transpose(0, 2, 1, 3)
    ).reshape(NCORES, 128, SL * 65)

    return [
        {
            "qt": qt[c * SL : (c + 1) * SL],
            "lkt": lkt[c * SL : (c + 1) * SL],
            "skt": skt[c * SL : (c + 1) * SL],
            "gkt": gktj[c],
            "lv": lvp[c * SL : (c + 1) * SL],
            "sv": svp[c * SL : (c + 1) * SL],
            "gv": gvj[c],
        }
        for c in range(NCORES)
    ]


_NC_CACHE = {}
LAST_RESULTS = None


def kernel(**inputs):
    global LAST_RESULTS
    if "nc" not in _NC_CACHE:
        _NC_CACHE["nc"] = _build_bass()
    nc = _NC_CACHE["nc"]
    in_maps = _prepare(inputs)
    res = run_bass_kernel_spmd(nc, in_maps, core_ids=list(range(NCORES)))
    LAST_RESULTS = res
    out = np.empty((NH, T, D), np.float32)
    for c in range(NCORES):
        # o is [SL, PPS, 128 q, ctxA|ZA|ctxB|ZB]; normalize + reorder to
        # [SL, T, D] on the host
        ot = res.results[c]["o"]
        ab = ot.reshape(SL, PPS // 2, 128, 4, 65).transpose(0, 1, 3, 2, 4)
        ab = ab.reshape(SL, T, 65)
        out[c * SL : (c + 1) * SL] = ab[:, :, 0:64] / ab[:, :, 64:65]
    return out.reshape(N, H, T, D)
